# revision 88
# baseline (speedup 1.0000x reference)
"""Distributed Trainium2 kernel for a contextual-loss module (raw Bass SPMD).

Math (per batch b, with y,x in [c=256, n=1024] layout, n = h*w):
    yn = y / ||y||_c ; xn = x / ||x||_c
    u  = yn^T @ xn                      (cosine similarity, [n, n])
    dist = 1 - u  (clip(0,2) never binds for randn inputs)
    dmin_j = max(1 - max_m u_jm, EPS)
    w = exp((1 - dist/dmin)/0.1) = exp(alpha_j * u'' + beta_j)   where
        u'' = y^T @ xn  (rows unnormalized),  r_j = 1/dmin_j,
        alpha_j = 10 * r_j / ||y_j||,  beta_j = 10 - 10 * r_j
    row max of w == 1 (exact whenever dmin > EPS; true with 200x margin
    for this data: min dmin = 2.1e-3), so
    cx_i_j = 1 / (sum_m w_jm + EPS)
    loss = mean_b(-log(mean_j cx_i_j + EPS))

Sharding: pure data parallel over batch, 8 batches per core on 8 cores.
Each core emits its partial of sum(-log(...))/64; the host adds the 8
partials (equivalent to the all-reduce of the scalar mean).

Engine split per batch:
    sync  : DMA y,x ([128, 4KB] contiguous descriptors)
    gpsimd: f32->bf16 casts, bf16 squares, x-normalize multiply
    tensor: ones-matmul partition reductions for ||x|| (replicated) and
            per-row-tile ||y|| columns, main y^T@xn matmuls, final
            cross-partition reduction of cx_i
    scalar: 1/sqrt via exp(-0.5*ln(.)) (Rsqrt ACT table is banned; ln+exp
            live in one table set with the main exp), main exp with
            per-partition scale/bias and fused row-sum (accum_out),
            final log
    vector: row-max over PSUM, small alpha/beta chains, reciprocals

Raw Bass (not Tile): this container's walrus rejects instructions with
multiple attached sync waits, so every wait is a standalone wait_ge.
Thresholds are precomputed with a counting pass, then emitted.
"""

import numpy as np

N_CORES = 8
B_LOC = 8          # batches per core
C = 256
N = 1024
P = 128
NT = N // P        # 8 row tiles
NCH = C // P       # 2 contraction chunks
EPS = 1e-5

_cache = {}


class _Em:
    """Per-engine emitter: pass 1 counts sem values, pass 2 emits.

    Only DMA ops carry per-op increments (+16, HWDGE convention). For the
    compute engines an increment is attached only at mark() points — the
    only values anyone waits on — which keeps sem-inc traffic sparse.
    """

    def __init__(self, counting, engine, sems, cnt, marks, requested):
        self.counting = counting
        self.engine = engine
        self.sems = sems
        self.cnt = cnt
        self.marks = marks
        self.requested = requested
        self.last = None

    def wait(self, sem, label):
        if self.counting:
            self.requested.add(label)
            return
        if label not in self.marks:
            return  # b<0 dependency: nothing to wait on
        self.engine.wait_ge(self.sems[sem], self.marks[label])

    def do(self, sem, fn, by=1):
        if sem == "dma":
            self.cnt[sem] = self.cnt.get(sem, 0) + by
        if not self.counting:
            ins = fn(self.engine)
            if sem == "dma":
                ins.then_inc(self.sems[sem], by)
            self.last = ins

    def mark(self, label, sem):
        if sem == "dma":
            if self.counting:
                assert label not in self.marks, f"duplicate mark {label}"
                self.marks[label] = self.cnt.get(sem, 0)
            return
        self.cnt[sem] = self.cnt.get(sem, 0) + 1
        if self.counting:
            assert label not in self.marks, f"duplicate mark {label}"
            self.marks[label] = self.cnt[sem]
        else:
            assert self.last is not None
            self.last.then_inc(self.sems[sem], 1)
            self.last = None


def _build():
    from contextlib import ExitStack

    import concourse.bass as bass
    import concourse.mybir as mybir

    f32 = mybir.dt.float32
    bf16 = mybir.dt.bfloat16
    AX = mybir.AxisListType
    OP = mybir.AluOpType
    AF = mybir.ActivationFunctionType

    import os

    debug = os.environ.get("KDEBUG") == "1"

    nc = bass.Bass()

    y_ext = nc.dram_tensor("y_feat", [B_LOC, C, N], f32, kind="ExternalInput")
    x_ext = nc.dram_tensor("x_feat", [B_LOC, C, N], f32, kind="ExternalInput")
    out_ext = nc.dram_tensor("out", [1, 1], f32, kind="ExternalOutput")
    if debug:
        dbg_ext = {
            "dbg_cx": nc.dram_tensor("dbg_cx", [P, B_LOC * NT], f32,
                                     kind="ExternalOutput"),
            "dbg_smax": nc.dram_tensor("dbg_smax", [P, NT], f32,
                                       kind="ExternalOutput"),
            "dbg_nyinv": nc.dram_tensor("dbg_nyinv", [P, NT], f32,
                                        kind="ExternalOutput"),
            "dbg_alpha": nc.dram_tensor("dbg_alpha", [P, NT], f32,
                                        kind="ExternalOutput"),
            "dbg_beta": nc.dram_tensor("dbg_beta", [P, NT], f32,
                                       kind="ExternalOutput"),
            "dbg_sall": nc.dram_tensor("dbg_sall", [P, NT], f32,
                                       kind="ExternalOutput"),
            "dbg_nxinv": nc.dram_tensor("dbg_nxinv", [P, N], f32,
                                        kind="ExternalOutput"),
            "dbg_u": nc.dram_tensor("dbg_u", [P, N], f32,
                                    kind="ExternalOutput"),
            "dbg_csum": nc.dram_tensor("dbg_csum", [1, B_LOC], f32,
                                       kind="ExternalOutput"),
            "dbg_umax": nc.dram_tensor("dbg_umax", [P, NT], f32,
                                       kind="ExternalOutput"),
            "dbg_dmin": nc.dram_tensor("dbg_dmin", [P, NT], f32,
                                       kind="ExternalOutput"),
            "dbg_a10": nc.dram_tensor("dbg_a10", [P, NT], f32,
                                      kind="ExternalOutput"),
            "dbg_sallall": nc.dram_tensor("dbg_sallall", [P, B_LOC * NT], f32,
                                          kind="ExternalOutput"),
            "dbg_ab": nc.dram_tensor("dbg_ab", [P, B_LOC * 4], f32,
                                     kind="ExternalOutput"),
        }

    with ExitStack() as ctx:
        sb = lambda nm, shape, dt: ctx.enter_context(nc.sbuf_tensor(nm, shape, dt))
        ps = lambda nm, shape, dt: ctx.enter_context(nc.psum_tensor(nm, shape, dt))
        sb2 = lambda nm, shape, dt: [sb(f"{nm}{i}", shape, dt) for i in range(2)]

        # double-buffered per-batch tensors (slot = b % 2)
        y_f = sb2("y_f", [P, NCH, N], f32)
        x_f = sb2("x_f", [P, NCH, N], f32)
        y_b = sb2("y_b", [P, NCH, N], bf16)
        x_b = sb2("x_b", [P, NCH, N], bf16)
        y2 = sb2("y2_", [P, NCH, N], bf16)
        y2s = sb2("y2s", [P, N], bf16)
        x2 = sb2("x2_", [P, NCH, N], bf16)
        xn = sb2("xn_", [P, NCH, N], bf16)
        nxinv = sb2("nxinv", [P, N], bf16)
        nyinv = sb2("nyinv", [P, NT], f32)
        nyneg = sb2("nyneg", [P, NT], f32)
        # Stride-8 "wide" layout for all per-row-tile scalars: tile t's
        # value lives at column 8*t, so every DVE slice is 32B-aligned.
        # (DVE reads at 4B/8B offsets return garbage when GpSimd streams
        # through the shared SBUF port; 32B-aligned reads are clean.)
        NP_ = NT // 2
        wide = lambda nm: sb2(nm, [P, NT * 8], f32)
        smax_w = wide("smaxw")
        dmin_w = wide("dminw")
        tdm_w = wide("tdmw")
        a10_w = wide("a10w")
        alpha_w = wide("alphaw")
        beta_w = wide("betaw")
        s_w = wide("sw")
        negny_w = wide("negnyw")
        t_ln = sb("t_ln", [P, 512], f32)
        t_lny = sb("t_lny", [P, NT], f32)
        t_cx = sb("t_cx", [P, NT], f32)
        ln10_b = sb("ln10_b", [P, 1], f32)
        ten_b = sb("ten_b", [P, 1], f32)
        lnyb_w = wide("lnybw")
        junk = sb("junk", [P, 1], f32)

        col8 = lambda T, t: T[:, 8 * t:8 * t + 1]
        # [P, 2, 1] strided view of pair k (columns 16k and 16k+8)
        vpair = lambda T, k: T[:].rearrange("p (t e) -> p t e", e=8)[
            :, 2 * k:2 * k + 2, 0:1]
        vall = lambda T: T[:].rearrange("p (t e) -> p t e", e=8)[:, :, 0:1]
        w_scr = sb("w_scr", [P, N], bf16)
        cx_all = sb("cx_all", [P, B_LOC * NT], f32)
        ones_w = sb("ones_w", [P, P], bf16)
        ones_col = sb("ones_col", [P, 1], bf16)
        ones_f32 = sb("ones_f32", [P, 1], f32)
        eps_b = sb("eps_b", [P, 1], f32)
        csum = sb("csum", [1, B_LOC], f32)
        lnb = sb("lnb", [1, B_LOC], f32)
        lsum = sb("lsum", [1, 1], f32)
        partial = sb("partial", [1, 1], f32)
        if debug:
            dbg_u_sb = sb("dbg_u_sb", [P, N], f32)
            dbg_nxinv_sb = sb("dbg_nxinv_sb", [P, N], f32)
            dbg_sallall_sb = sb("dbg_sallall_sb", [P, B_LOC * NT], f32)
            dbg_ab_sb = sb("dbg_ab_sb", [P, B_LOC * 4], f32)

        # PSUM: 3x u (2 banks each) + nx (1 bank) + small (1 bank) = 8 banks
        u_ps = [ps(f"u_ps{i}", [P, N], f32) for i in range(3)]
        nx_ps = ps("nx_ps", [P, 512], f32)
        small_ps = ps("small_ps", [P, 64], f32)

        sems = {
            "dma": ctx.enter_context(nc.semaphore("dma_sem")),
            "gp": ctx.enter_context(nc.semaphore("gp_sem")),
            "te": ctx.enter_context(nc.semaphore("te_sem")),
            "act": ctx.enter_context(nc.semaphore("act_sem")),
            "dve": ctx.enter_context(nc.semaphore("dve_sem")),
        }

        # Bass(target_bir_lowering=False) skips the init-time semaphore
        # clear, so sems carry values from previous NEFF executions and
        # every wait_ge threshold would be wrong. Clear them explicitly,
        # then an NRT-level barrier (outside the bass sem range) keeps the
        # other engines from racing ahead of the clear.
        from concourse.bass import compact_to_ranges

        for sem_range in compact_to_ranges(
            [s for s in nc._kernel_sem_range if s not in nc.barrier_sems]
        ):
            nc.gpsimd.dma_reset(sem_range)
            nc.gpsimd.sem_clear(sem_range)
        nc._nrt_pseudo_barrier()

        # ---------------- engine programs ----------------

        def prog_sync(E):
            for b in range(B_LOC):
                s = b % 2
                E.wait("dve", f"dve_cast_{b - 2}")
                for c in range(NCH):
                    E.do("dma", lambda e, s=s, b=b, c=c: e.dma_start(
                        y_f[s][:, c, :], y_ext[b, c * P:(c + 1) * P, :]), by=16)
                    E.mark(f"dma_y{c}_{b}", "dma")
                for c in range(NCH):
                    E.do("dma", lambda e, s=s, b=b, c=c: e.dma_start(
                        x_f[s][:, c, :], x_ext[b, c * P:(c + 1) * P, :]), by=16)
                    E.mark(f"dma_x{c}_{b}", "dma")
            E.wait("dve", "dve_final")
            E.do("dma", lambda e: e.dma_start(out_ext[:, :], partial[:]), by=16)
            if debug:
                s1 = (B_LOC - 1) % 2
                items = [("dbg_cx", cx_all[:]),
                         ("dbg_sall", vall(s_w[s1])),
                         ("dbg_nxinv", dbg_nxinv_sb[:]),
                         ("dbg_u", dbg_u_sb[:]),
                         ("dbg_csum", csum[:]),
                         ("dbg_sallall", dbg_sallall_sb[:]),
                         ("dbg_ab", dbg_ab_sb[:]),
                         ("dbg_smax", vall(smax_w[s1])),
                         ("dbg_nyinv", nyinv[s1][:]),
                         ("dbg_alpha", vall(alpha_w[s1])),
                         ("dbg_beta", vall(beta_w[s1])),
                         ("dbg_dmin", vall(dmin_w[s1])),
                         ("dbg_a10", vall(a10_w[s1]))]
                for nm, src in items:
                    def dbg_dma(e, nm=nm, src=src):
                        with nc.allow_non_contiguous_dma(reason="debug dump"):
                            return e.dma_start(dbg_ext[nm][:], src)
                    E.do("dma", dbg_dma, by=16)

        def prog_gpsimd(E):
            E.do("gp", lambda e: e.memset(ones_w[:], 1.0))
            E.do("gp", lambda e: e.memset(ones_col[:], 1.0))
            E.do("gp", lambda e: e.memset(ones_f32[:], 1.0))
            E.do("gp", lambda e: e.memset(eps_b[:], EPS))
            E.do("gp", lambda e: e.memset(ln10_b[:], float(np.log(10.0))))
            E.do("gp", lambda e: e.memset(ten_b[:], 10.0))
            for b in range(B_LOC):
                s = b % 2
                # casts y_b/x_b now live on DVE (6x faster there)
                E.wait("dve", f"dve_cast_{b}")
                for c in range(NCH):
                    E.do("gp", lambda e, s=s, c=c: e.tensor_mul(
                        y2[s][:, c, :], y_b[s][:, c, :], y_b[s][:, c, :]))
                for c in range(NCH):
                    E.do("gp", lambda e, s=s, c=c: e.tensor_mul(
                        x2[s][:, c, :], x_b[s][:, c, :], x_b[s][:, c, :]))
                # pre-sum the y^2 chunks so ||y|| needs one matmul per tile
                E.do("gp", lambda e, s=s: e.tensor_add(
                    y2s[s][:], y2[s][:, 0, :], y2[s][:, 1, :]))
                E.mark(f"gp_x2_{b}", "gp")
                E.wait("act", f"act_nxinv_{b}")
                for c in range(NCH):
                    E.do("gp", lambda e, s=s, c=c: e.tensor_mul(
                        xn[s][:, c, :], x_b[s][:, c, :], nxinv[s][:]))
                E.mark(f"gp_xn_{b}", "gp")

        def prog_tensor(E):
            def norms_te(E, b):
                s = b % 2
                E.wait("gp", f"gp_x2_{b}")
                # nx h0 first, then the 16 ny matmuls absorb the wait for
                # ACT's ln of h0 before the h1 matmuls need the psum bank
                E.wait("act", f"act_lnh1_{b - 1}")
                for c in range(NCH):
                    E.do("te" if c == NCH - 1 else None,
                         lambda e, s=s, c=c: e.matmul(
                             nx_ps[:], ones_w[:],
                             x2[s][:, c, 0:512],
                             start=(c == 0), stop=(c == NCH - 1)))
                E.mark(f"te_nxh0_{b}", "te")
                # ||y||^2 columns [128, NT] in small_ps[:, 0:NT]
                E.wait("act", f"act_lnny_{b - 1}")
                for t in range(NT):
                    E.do("te" if t == NT - 1 else None,
                         lambda e, s=s, t=t: e.matmul(
                             small_ps[:, t:t + 1],
                             y2s[s][:, t * P:(t + 1) * P],
                             ones_col[:],
                             start=True, stop=True))
                E.mark(f"te_ny_{b}", "te")
                E.wait("act", f"act_lnh0_{b}")
                for c in range(NCH):
                    E.do("te" if c == NCH - 1 else None,
                         lambda e, s=s, c=c: e.matmul(
                             nx_ps[:], ones_w[:],
                             x2[s][:, c, 512:1024],
                             start=(c == 0), stop=(c == NCH - 1)))
                E.mark(f"te_nxh1_{b}", "te")

            norms_te(E, 0)
            for b in range(B_LOC):
                s = b % 2
                # main tiles; batch b+1's norm matmuls are hoisted into the
                # middle so the prep chain overlaps these mains
                E.wait("gp", f"gp_xn_{b}")
                for t in range(NT):
                    g = b * NT + t
                    if g - 3 < 3:
                        E.wait("dve", f"dve_red_{g - 3}")
                    # act_exp(g-3) transitively implies dve_red(g-3)
                    E.wait("act", f"act_exp_{g - 3}")
                    for c in range(NCH):
                        for h in range(2):
                            E.do("te" if (c == NCH - 1 and h == 1) else None,
                                 lambda e, s=s, t=t, c=c, h=h, g=g: e.matmul(
                                     u_ps[g % 3][:, h * 512:(h + 1) * 512],
                                     y_b[s][:, c, t * P:(t + 1) * P],
                                     xn[s][:, c, h * 512:(h + 1) * 512],
                                     start=(c == 0), stop=(c == NCH - 1)))
                    E.mark(f"te_main_{g}", "te")
                    if t == 4 and b + 1 < B_LOC:
                        norms_te(E, b + 1)
            # final partition-reduction of cx_i
            E.wait("act", f"act_cx_{B_LOC - 1}")
            E.do("te", lambda e: e.matmul(
                small_ps[:1, :], ones_f32[:], cx_all[:], start=True, stop=True))
            E.mark("te_loss", "te")

        def prog_scalar(E):
            def norms_act(E, b):
                s = b % 2
                # WAR on nxinv slot vs gp xn readers of b-2
                E.wait("gp", f"gp_xn_{b - 2}")
                for h in range(2):
                    E.wait("te", f"te_nxh{h}_{b}")
                    E.do("act", lambda e: e.activation(t_ln[:], nx_ps[:], AF.Ln))
                    E.mark(f"act_lnh{h}_{b}", "act")
                    E.do("act", lambda e, s=s, h=h: e.activation(
                        nxinv[s][:, h * 512:(h + 1) * 512], t_ln[:],
                        AF.Exp, scale=-0.5))
                E.mark(f"act_nxinv_{b}", "act")
                # WAR on nyinv/t_lny slots vs dve readers of b-2
                E.wait("dve", f"dve_nyprep_{b - 2}")
                E.wait("te", f"te_ny_{b}")
                E.do("act", lambda e: e.activation(
                    t_lny[:], small_ps[:, 0:NT], AF.Ln))
                E.mark(f"act_lnny_{b}", "act")
                E.do("act", lambda e, s=s: e.activation(
                    nyinv[s][:], t_lny[:], AF.Exp, scale=-0.5))
                E.mark(f"act_ny_{b}", "act")

            norms_act(E, 0)
            for b in range(B_LOC):
                s = b % 2
                for k in range(NP_):
                    # whole temperature chain on ACT (no DVE round trip):
                    # a10 = exp(ln10 - ln(dmin)); alpha = exp(ln10 + ln(nyinv)
                    # - ln(dmin)) via per-tile bias; beta = 10 - a10
                    E.wait("dve", f"dve_dmin_{b}_{k}")
                    E.do("act", lambda e, s=s, k=k: e.activation(
                        vpair(tdm_w[s], k), vpair(dmin_w[s], k), AF.Ln))
                    E.do("act", lambda e, s=s, k=k: e.activation(
                        vpair(a10_w[s], k), vpair(tdm_w[s], k), AF.Exp,
                        scale=-1.0, bias=ln10_b[:]))
                    for t in (2 * k, 2 * k + 1):
                        E.do("act", lambda e, s=s, t=t: e.activation(
                            col8(alpha_w[s], t), col8(tdm_w[s], t), AF.Exp,
                            scale=-1.0, bias=col8(lnyb_w[s], t)))
                        E.do("act", lambda e, s=s, t=t: e.activation(
                            col8(beta_w[s], t), col8(a10_w[s], t), AF.Identity,
                            scale=-1.0, bias=ten_b[:]))
                    E.mark(f"act_a10_{b}_{k}", "act")
                    if debug and k == 0:
                        E.wait("dve", f"dve_chain_{b}_{k}")
                        E.do("act", lambda e, s=s, b=b: e.activation(
                            dbg_ab_sb[:, b * 4:b * 4 + 1], col8(alpha_w[s], 0),
                            AF.Identity))
                        E.do("act", lambda e, s=s, b=b: e.activation(
                            dbg_ab_sb[:, b * 4 + 2:b * 4 + 3], col8(alpha_w[s], 1),
                            AF.Identity))
                        E.do("act", lambda e, s=s, b=b: e.activation(
                            dbg_ab_sb[:, b * 4 + 3:b * 4 + 4], col8(beta_w[s], 1),
                            AF.Identity))
                    for t in (2 * k, 2 * k + 1):
                        g = b * NT + t
                        E.do("act", lambda e, s=s, t=t, g=g: e.activation(
                            w_scr[:], u_ps[g % 3][:], AF.Exp,
                            bias=col8(beta_w[s], t),
                            scale=col8(alpha_w[s], t),
                            accum_out=col8(s_w[s], t)))
                        E.mark(f"act_exp_{g}", "act")
                    if k == 1 and b + 1 < B_LOC:
                        # hoisted: next batch's norm ln/exp overlaps this
                        # batch's last main tiles (must sit before pair 2 so
                        # its TE dependencies close before TE's t==4 insert)
                        norms_act(E, b + 1)
                # cx_i = 1/(S+EPS) via exp(-ln(S+EPS)) for the whole batch.
                # Spacer first: exp(t=7)'s accum_out into s_all commits after
                # the main output stream; a distance-0 ACT read sees stale data.
                E.do("act", lambda e: e.activation(junk[:], junk[:], AF.Identity))
                if debug:
                    E.do("act", lambda e, s=s, b=b: e.activation(
                        dbg_sallall_sb[:, b * NT:(b + 1) * NT].rearrange(
                            "p (t e) -> p t e", e=1),
                        vall(s_w[s]), AF.Identity))
                E.do("act", lambda e, s=s: e.activation(
                    t_cx[:].rearrange("p (t e) -> p t e", e=1),
                    vall(s_w[s]), AF.Ln, bias=eps_b[:]))
                E.do("act", lambda e, b=b: e.activation(
                    cx_all[:, b * NT:(b + 1) * NT], t_cx[:], AF.Exp, scale=-1.0))
                if b == B_LOC - 1:
                    # spacer so the TE loss-matmul's operand fetch doesn't race
                    # the tail of the cx_all write
                    E.do("act", lambda e: e.activation(junk[:], junk[:],
                                                       AF.Identity))
                E.mark(f"act_cx_{b}", "act")
            # final log
            E.wait("dve", "dve_csum")
            E.do("act", lambda e: e.activation(
                lnb[:], csum[:], AF.Ln, scale=1.0 / N, bias=eps_b[:1, :]))
            E.mark("act_lnb", "act")

        def prog_vector(E):
            # DVE constraints baked into this schedule (all verified on HW):
            #  - no 2-tensor DVE ops (GpSimd port contention corrupts them)
            #  - every DVE slice is 32B-aligned (stride-8 wide layout)
            #  - >=1 op between a DVE producer and DVE consumer (stale-read)
            #  - chain_k must be marked before red(2k+3) (PSUM-reuse cycle)
            def J(E):
                E.do("dve", lambda e: e.tensor_scalar_mul(junk[:], junk[:], 1.0))

            def casts(E, b):
                # f32 -> bf16 casts (DVE copy runs in 2x mode; much faster
                # than GpSimd CAST). Slot WAR: TE mains of b-2 read y_b/xn.
                sc = b % 2
                E.wait("te", f"te_main_{(b - 2) * NT + NT - 1}")
                for c in range(NCH):
                    E.wait("dma", f"dma_y{c}_{b}")
                    E.do("dve", lambda e, sc=sc, c=c: e.tensor_copy(
                        y_b[sc][:, c, :], y_f[sc][:, c, :]))
                for c in range(NCH):
                    E.wait("dma", f"dma_x{c}_{b}")
                    E.do("dve", lambda e, sc=sc, c=c: e.tensor_copy(
                        x_b[sc][:, c, :], x_f[sc][:, c, :]))
                E.mark(f"dve_cast_{b}", "dve")

            casts(E, 0)
            for b in range(B_LOC):
                s = b % 2
                if b + 1 < B_LOC:
                    casts(E, b + 1)
                E.wait("act", f"act_ny_{b}")
                # -nyinv and (ln10 - 0.5*ln(Ny^2)) straight into the
                # 32B-aligned wide layouts (strided DVE writes are fine)
                E.do("dve", lambda e, s=s: e.tensor_scalar_mul(
                    vall(negny_w[s]), nyinv[s][:].rearrange(
                        "p (t e) -> p t e", e=1), -1.0))
                E.do("dve", lambda e, s=s: e.tensor_scalar(
                    vall(lnyb_w[s]), t_lny[:].rearrange(
                        "p (t e) -> p t e", e=1), -0.5,
                    float(np.log(10.0)), op0=OP.mult, op1=OP.add))
                J(E)
                E.mark(f"dve_nyprep_{b}", "dve")
                for t in range(NT):
                    g = b * NT + t
                    k = t // 2
                    E.wait("te", f"te_main_{g}")
                    E.do("dve", lambda e, s=s, t=t, g=g: e.tensor_reduce(
                        col8(smax_w[s], t), u_ps[g % 3][:],
                        axis=AX.X, op=OP.max))
                    E.mark(f"dve_red_{g}", "dve")
                    if t % 2 == 1:
                        # dmin = 1 - smax*nyinv, clamped to EPS (the clamp is
                        # load-bearing: dmin can reach 2e-3 and bf16 noise in
                        # u could push it negative -> Ln would NaN)
                        for tt in (t - 1, t):
                            E.do("dve", lambda e, s=s, tt=tt: e.tensor_scalar(
                                col8(dmin_w[s], tt), col8(smax_w[s], tt),
                                col8(negny_w[s], tt), 1.0,
                                op0=OP.mult, op1=OP.add))
                        J(E)
                        E.do("dve", lambda e, s=s, k=k: e.tensor_scalar_max(
                            vpair(dmin_w[s], k), vpair(dmin_w[s], k), EPS))
                        E.mark(f"dve_dmin_{b}_{k}", "dve")
            # final
            E.wait("te", "te_loss")
            E.do("dve", lambda e: e.tensor_reduce(
                csum[:], small_ps[:1, :].rearrange("p (b t) -> p b t", t=NT),
                axis=AX.X, op=OP.add))
            J(E)
            E.mark("dve_csum", "dve")
            E.wait("act", "act_lnb")
            E.do("dve", lambda e: e.tensor_reduce(
                lsum[:], lnb[:], axis=AX.X, op=OP.add))
            J(E)
            E.do("dve", lambda e: e.tensor_scalar_mul(
                partial[:], lsum[:], -1.0 / (B_LOC * N_CORES)))
            J(E)
            if debug:
                # u tile (b=7, t=7) still lives in u_ps[63 % 3] = u_ps[0]
                E.do("dve", lambda e: e.tensor_copy(dbg_u_sb[:], u_ps[0][:]))
                E.do("dve", lambda e: e.tensor_copy(
                    dbg_nxinv_sb[:], nxinv[(B_LOC - 1) % 2][:]))
            E.mark("dve_final", "dve")

        # ---------------- two passes ----------------
        progs = {
            "sync": prog_sync,
            "gpsimd": prog_gpsimd,
            "tensor": prog_tensor,
            "scalar": prog_scalar,
            "vector": prog_vector,
        }
        marks = {}
        requested = set()
        for name, prog in progs.items():
            prog(_Em(True, None, sems, {}, marks, requested))
        for lbl in requested:
            if lbl not in marks:
                assert "-" in lbl, f"waited label {lbl} never marked"

        with nc.Block() as block:
            @block.sync
            def _(eng):
                prog_sync(_Em(False, eng, sems, {}, marks, requested))

            @block.gpsimd
            def _(eng):
                prog_gpsimd(_Em(False, eng, sems, {}, marks, requested))

            @block.tensor
            def _(eng):
                prog_tensor(_Em(False, eng, sems, {}, marks, requested))

            @block.scalar
            def _(eng):
                prog_scalar(_Em(False, eng, sems, {}, marks, requested))

            @block.vector
            def _(eng):
                prog_vector(_Em(False, eng, sems, {}, marks, requested))

    return nc


def _ensure_ntff_hook():
    """This image's antenv package lacks axon_hooks; bass_utils imports it
    unconditionally when BASS_TRACE is set. Recreate it from the boot
    module's ctypes implementation so tracing works."""
    import sys
    import types

    if "antenv.axon_hooks" not in sys.modules:
        mod = types.ModuleType("antenv.axon_hooks")
        box = [None]

        def set_axon_ntff_profile_hook(h):
            box[0] = h

        def get_axon_ntff_profile_hook():
            if box[0] is None:
                try:
                    from trn_agent_boot.trn_boot import _ntff_profile_via_ctypes

                    box[0] = _ntff_profile_via_ctypes("/opt/axon/libaxon_pjrt.so")
                except Exception:
                    return None
            return box[0]

        mod.set_axon_ntff_profile_hook = set_axon_ntff_profile_hook
        mod.get_axon_ntff_profile_hook = get_axon_ntff_profile_hook
        sys.modules["antenv.axon_hooks"] = mod
        try:
            import antenv

            antenv.axon_hooks = mod
        except Exception:
            pass
    import concourse.bass_utils as bu

    bu.upload_artifacts = lambda tmpdir: str(tmpdir)  # zero-egress container


def kernel(y_feat: np.ndarray, x_feat: np.ndarray) -> np.ndarray:
    _ensure_ntff_hook()
    from concourse.bass_utils import run_bass_kernel_spmd

    if "nc" not in _cache:
        _cache["nc"] = _build()
    nc = _cache["nc"]

    y = np.ascontiguousarray(np.asarray(y_feat, np.float32).reshape(64, C, N))
    x = np.ascontiguousarray(np.asarray(x_feat, np.float32).reshape(64, C, N))
    in_maps = [
        {"y_feat": y[i * B_LOC:(i + 1) * B_LOC], "x_feat": x[i * B_LOC:(i + 1) * B_LOC]}
        for i in range(N_CORES)
    ]
    res = run_bass_kernel_spmd(nc, in_maps, core_ids=list(range(N_CORES)))
    _cache["last_results"] = res
    total = np.float32(0.0)
    for r in res.results:
        total += np.float32(r["out"].reshape(-1)[0])
    return np.float32(total).reshape(())
